# revision 1
# baseline (speedup 1.0000x reference)
"""Multi-head attention (projections + causal/padded softmax attention + output
projection + residual + LayerNorm) as a Bass/Tile kernel on 8 Trainium2 cores.

Sharding: tensor-parallel over heads within each batch. Core c handles batch
g = c // 4 and heads [4*(c%4), 4*(c%4)+4). Each core projects Q/K/V for its
4 heads over the full sequence, runs causal attention in a transposed layout
(scoresT[key, row]), and produces ctxT[dh, row]. One 8-way AllToAll per
head-pair redistributes ctxT with a fully STATIC slot map: slot j carries rows
[j*256, (j+1)*256) of the sender's batch, so core j ends up owning that row
range of BOTH batches (cores 0-3 receive batch-0 contributions from cores 0-3
and batch-1 contributions from cores 4-7 in distinct sender slots). No runtime
core-id addressing, no barriers: Tile orders staging DMAs before each
collective and the pair-0 collective overlaps pair-1's attention.

Layout trick: all matmul operands are pre-transposed/pre-cast on the host
(numpy) so every DMA is contiguous: qT/kT/vT = x^T as bf16, WqT/WkT/WvT/WoT =
W^T as bf16. The PE contracts over partitions, so the contraction dim (d_model
or d_head) always sits on the partition axis.

Softmax: scores are bounded (|s| ~ 5) so exp is computed without max
subtraction; both heads' scores share one 2-bank psum tile so a single
scalar-engine exp(scale*s + pad_bias) covers them, with the padding mask
folded into the per-key bias. The causal boundary adds a precomputed
triangular -1e9 bias onto the diagonal 128-col band (vector engine) before
exp. The denominator comes from augmenting V with a ones column (row dh of
ctxT psum = sum of probs); the divide uses a [1,R] fast-approx reciprocal
(the DVE RECIPROCAL op costs a flat ~3.3us) + partition broadcast + multiply.

PE p-state discipline: the TRN2 tensor engine runs at 1.2GHz until it has
been continuously busy ~3us (then 2.4GHz), so every stall halves throughput.
Attention is software-pipelined (ctx matmul of chunk kb-3 emitted between the
score matmuls of chunk kb), and all projection work that the first attention
row-range doesn't need (K slice 1, V chunks 4+, Q slices 1-3) is deferred and
interleaved into the first pair's attention stream as scalar-independent PE
filler. Weights load on the scalar DMA queue, P3 constants load after P1's
input stream, and a dummy warm-up collective absorbs the ~11us first-use
latency of the NRT collective stream.

PSUM budget (8 banks): sc=3x2 banks + ctx=2x1 = 8; the sc tag doubles as the
projection and Wo accumulators. A no-sync scheduler fence keeps the cc-gated
fetch DMAs from blocking the staging DMAs on the in-order sync queue.
"""

import math
from contextlib import ExitStack

import numpy as np
import ml_dtypes

import concourse.bass as bass
import concourse.mybir as mybir
import concourse.tile as tile
from concourse import bacc
from concourse.bass_utils import run_bass_kernel_spmd

BF16 = mybir.dt.bfloat16
F32 = mybir.dt.float32

NEG_INF = -1e9
LN_EPS = 1e-6


class Cfg:
    def __init__(self, B=2, S=2048, D=1024, H=16, dh=64, kmax=None):
        self.B, self.S, self.D, self.H, self.dh = B, S, D, H, dh
        # kmax: max(sen_len) — keys beyond are fully masked, so K/V
        # projection and the attention key loop stop at this bound.
        self.kmax = S if kmax is None else min(int(kmax), S)
        self.NC = 8                      # cores
        self.G = 4                       # cores per batch group
        self.HPC = H // self.G           # heads per core
        self.PAIRS = self.HPC // 2       # head pairs per core
        self.D4 = self.HPC * dh          # per-core projection width
        self.RQ = S // self.G            # rows per core in Wo/LN phase
        self.NR = 4                      # attention row ranges
        self.RNG = S // self.NR          # rows per range (== RQ)
        self.RSL = S // self.NC          # rows per A2A slot (256)
        self.DC = D // 128               # contraction chunks
        self.KCH = S // 128              # key chunks
        self.NS = max(1, S // 512)       # projection n-slices
        self.NSW = S // self.NS          # cols per n-slice
        self.WON = max(1, D // 512)      # Wo n-slices
        self.WONW = D // self.WON
        self.D4C = self.D4 // 128        # 128-chunks in per-core ctx width
        self.KB_MAX = -(-self.kmax // 128)          # key chunks actually used
        self.NS_K = -(-(self.KB_MAX * 128) // self.NSW)  # K-proj n-slices
        # Large kmax needs the SBUF for K/V state; drop the Q-interleave
        # buffers and run leaner pipelining in that case.
        self.LEAN = self.KB_MAX > 10
        # runtime-detected LN specializations (set by run() from the actual
        # inputs, so the program stays correct for arbitrary gamma/beta)
        self.G1 = False   # gamma == ones
        self.B0 = False   # beta == zeros
        assert self.RQ == self.RNG
        assert self.PAIRS >= 1 and self.HPC % 2 == 0


def build_program(cfg: Cfg):
    """Build the (SPMD-identical) Bass program."""
    nc = bacc.Bacc("TRN2", target_bir_lowering=False, debug=False,
                   num_devices=cfg.NC)

    S, D, dh = cfg.S, cfg.D, cfg.dh
    D4, RQ, RNG, RSL = cfg.D4, cfg.RQ, cfg.RNG, cfg.RSL

    # All inputs are pre-tiled on the host so every DMA is contiguous per
    # partition: x inputs as [ns, p, dc, cols], weights as [p, dc, outs].
    qT = nc.dram_tensor("qT", [cfg.NS, 128, cfg.DC, cfg.NSW], BF16,
                        kind="ExternalInput").ap()
    kT = nc.dram_tensor("kT", [cfg.NS_K, 128, cfg.DC, cfg.NSW], BF16,
                        kind="ExternalInput").ap()
    vT = nc.dram_tensor("vT", [cfg.KB_MAX, 128, cfg.DC, 128], BF16,
                        kind="ExternalInput").ap()
    wqT = nc.dram_tensor("wqT", [128, cfg.DC, D4], BF16,
                         kind="ExternalInput").ap()
    wkT = nc.dram_tensor("wkT", [128, cfg.DC, D4], BF16,
                         kind="ExternalInput").ap()
    wvT = nc.dram_tensor("wvT", [128, cfg.DC, D4], BF16,
                         kind="ExternalInput").ap()
    woT = nc.dram_tensor("woT", [128, cfg.DC, D], BF16,
                         kind="ExternalInput").ap()
    resid = nc.dram_tensor("resid", [128, cfg.G, D], F32,
                           kind="ExternalInput").ap()
    pad_bias = nc.dram_tensor("pad_bias", [128, cfg.KCH], F32,
                              kind="ExternalInput").ap()
    gamma = nc.dram_tensor("gamma", [1, D], BF16, kind="ExternalInput").ap()
    beta = nc.dram_tensor("beta", [1, D], BF16, kind="ExternalInput").ap()
    out_shard = nc.dram_tensor("out_shard", [RQ, D], BF16,
                               kind="ExternalOutput").ap()

    with tile.TileContext(nc) as tc, ExitStack() as ctx:
        consts = ctx.enter_context(tc.tile_pool(name="consts", bufs=1))
        xin = ctx.enter_context(tc.tile_pool(name="xin", bufs=2))
        proj = ctx.enter_context(tc.tile_pool(name="proj", bufs=1))
        att = ctx.enter_context(tc.tile_pool(name="att", bufs=2))
        small = ctx.enter_context(tc.tile_pool(name="small", bufs=2))
        lnp = ctx.enter_context(tc.tile_pool(name="lnp", bufs=2))
        ctxf = ctx.enter_context(tc.tile_pool(name="ctxf", bufs=1))
        dram = ctx.enter_context(
            tc.tile_pool(name="dram", bufs=1, space="DRAM"))
        psum = ctx.enter_context(
            tc.tile_pool(name="psum", bufs=1, space="PSUM"))

        # ---- prologue: all constants (incl. P3's, so P3 never waits) -------
        # Weights ride the scalar engine's DMA queue so the sync queue can
        # start streaming the K/Q/V activations immediately (parallel DMA).
        wq_sb = consts.tile([128, cfg.DC, D4], BF16)
        wk_sb = consts.tile([128, cfg.DC, D4], BF16)
        wv_sb = consts.tile([128, cfg.DC, D4], BF16)
        for w_sb, w_dram in ((wk_sb, wkT), (wv_sb, wvT), (wq_sb, wqT)):
            nc.scalar.dma_start(out=w_sb, in_=w_dram)

        pb_sb = consts.tile([128, cfg.KCH], F32)
        nc.scalar.dma_start(out=pb_sb, in_=pad_bias)

        # P3 constants (loaded after P1's input stream, see below)
        wo_sb = consts.tile([128, cfg.DC, D], BF16)
        g_row = consts.tile([1, D], BF16)
        b_row = consts.tile([1, D], BF16)
        gamma_bc = consts.tile([128, D], BF16)
        beta_bc = consts.tile([128, D], BF16)
        eps_sb = consts.tile([128, 1], F32)
        nc.vector.memset(eps_sb, LN_EPS)
        res_sb = consts.tile([128, cfg.G, D], F32)
        # causal triangle bias: tri[p, f] = 0 if f >= p else NEG_INF.
        # Added (by the vector engine) onto the diagonal 128-col band of the
        # scores before exp — keeps the gpsimd queue out of the PE's
        # dependency chain.
        tri = consts.tile([128, 128], F32)
        nc.vector.memset(tri, 0.0)
        nc.gpsimd.affine_select(
            out=tri, in_=tri, pattern=[[1, 128]], base=0,
            channel_multiplier=-1, compare_op=mybir.AluOpType.is_ge,
            fill=NEG_INF)

        # A2A buffers: one per head-pair; slot j = rows [j*RSL,(j+1)*RSL).
        a2a_in = [dram.tile([cfg.NC, 128, RSL], BF16, name=f"a2a_in{p}",
                            tag=f"a2a_in{p}") for p in range(cfg.PAIRS)]
        a2a_out = [dram.tile([cfg.NC, 128, RSL], BF16, name=f"a2a_out{p}",
                             tag=f"a2a_out{p}") for p in range(cfg.PAIRS)]

        # warm up the NRT collective stream during P1: the first collective
        # after the prelude barrier pays ~11us of trigger latency; a dummy
        # 4KB AllToAll absorbs it so cc0/cc1 start promptly.
        warm_in = dram.tile([cfg.NC, 128, 2], BF16, name="warm_in",
                            tag="warm_in")
        warm_out = dram.tile([cfg.NC, 128, 2], BF16, name="warm_out",
                             tag="warm_out")
        nc.gpsimd.collective_compute(
            "AllToAll", mybir.AluOpType.bypass,
            replica_groups=[list(range(cfg.NC))],
            ins=[warm_in[:]], outs=[warm_out[:]])

        # ---- P1: projections (K, V first so attention can start early) ----
        # K/V SBUF is sized to the kmax actually used, not full S.
        qhT_sb = proj.tile([128, cfg.PAIRS, S], BF16)
        khT_sb = proj.tile([128, cfg.PAIRS, cfg.NS_K * cfg.NSW], BF16)
        vh_sb = proj.tile([128, cfg.KB_MAX, cfg.HPC * (dh + 1)], BF16)

        def qk_proj(x_dram, w_sb, out_sb, ns_count=None, ns_start=0):
            for ns in range(ns_start,
                            ns_count if ns_count is not None else cfg.NS):
                x_ns = xin.tile([128, cfg.DC, cfg.NSW], BF16, tag="x_ns",
                                name="x_ns")
                nc.sync.dma_start(out=x_ns, in_=x_dram[ns])
                for pair in range(cfg.PAIRS):
                    ps = psum.tile([128, cfg.NSW], F32, tag="ctx", bufs=4,
                                   name="ps_pj")
                    for dc in range(cfg.DC):
                        nc.tensor.matmul(
                            ps, w_sb[:, dc, pair * 128:(pair + 1) * 128],
                            x_ns[:, dc, :],
                            start=dc == 0, stop=dc == cfg.DC - 1)
                    nc.vector.tensor_copy(
                        out=out_sb[:, pair, ns * cfg.NSW:(ns + 1) * cfg.NSW],
                        in_=ps)

        def v_chunk(kb):
            v_kb = xin.tile([128, cfg.DC, 128], BF16, tag="v_kb", bufs=3,
                            name="v_kb")
            nc.sync.dma_start(out=v_kb, in_=vT[kb])

            def mm(p, dc, psv):
                nc.tensor.matmul(psv, v_kb[:, dc, :], wv_sb[:, dc, :],
                                 start=dc == 0, stop=dc == cfg.DC - 1)
                if dc == cfg.DC - 1:
                    nc.vector.tensor_copy(
                        out=vh_sb[:, kb, :]
                        .rearrange("p (h e) -> p h e", e=dh + 1)[:, :, 0:dh],
                        in_=psv.rearrange("p (h e) -> p h e", e=dh))
                    nc.vector.memset(
                        vh_sb[:, kb, :]
                        .rearrange("p (h e) -> p h e", e=dh + 1)
                        [:, :, dh:dh + 1], 1.0)
            return [(None, dc, mm, D4) for dc in range(cfg.DC)]

        def xw_slice(ns, x_tile, w_sb, out_sb):
            def mm(p, dc, qp):
                nc.tensor.matmul(
                    qp, w_sb[:, dc, p * 128:(p + 1) * 128],
                    x_tile[:, dc, :], start=dc == 0, stop=dc == cfg.DC - 1)
                if dc == cfg.DC - 1:
                    nc.vector.tensor_copy(
                        out=out_sb[:, p, ns * cfg.NSW:(ns + 1) * cfg.NSW],
                        in_=qp)
            return [(p, dc, mm, cfg.NSW)
                    for p in range(cfg.PAIRS) for dc in range(cfg.DC)]

        class Job:
            """A deferred stream of projection matmuls used as PE filler
            inside the attention loop (keeps the p-state ramp alive)."""
            def __init__(self, items):
                self.items = items
                self.idx = 0
                self.ps = None

            def emit(self, n):
                for _ in range(n):
                    if self.idx >= len(self.items):
                        return
                    p, dc, mm, width = self.items[self.idx]
                    self.idx += 1
                    if dc == 0:
                        self.ps = psum.tile([128, width], F32, tag="ctx",
                                            bufs=4, name="fill_ps")
                    mm(p, dc, self.ps)

            def done(self):
                return self.idx >= len(self.items)

            def finish(self):
                self.emit(len(self.items) - self.idx)

        # P1 proper: K slice 0, V chunks 0..3, Q slice 0 — the minimum the
        # first attention row-range needs. The rest becomes filler.
        # The very first x transfer is split in half so the first matmul
        # starts ~2us sooner (it only waits on 512KB, not 1MB).
        x_k0 = xin.tile([128, cfg.DC, cfg.NSW], BF16, tag="x_ns",
                        name="x_k0")
        hw_ = cfg.NSW // 2
        nc.sync.dma_start(out=x_k0[:, :, 0:hw_], in_=kT[0][:, :, 0:hw_])
        nc.sync.dma_start(out=x_k0[:, :, hw_:], in_=kT[0][:, :, hw_:])
        for pair in range(cfg.PAIRS):
            for half in range(2):
                ps = psum.tile([128, cfg.NSW], F32, tag="ctx", bufs=4,
                               name="ps_k0")
                cs = slice(half * hw_, half * hw_ + hw_)
                for dc in range(cfg.DC):
                    nc.tensor.matmul(
                        ps[:, 0:hw_],
                        wk_sb[:, dc, pair * 128:(pair + 1) * 128],
                        x_k0[:, dc, cs],
                        start=dc == 0, stop=dc == cfg.DC - 1)
                nc.vector.tensor_copy(
                    out=khT_sb[:, pair, half * hw_:half * hw_ + hw_],
                    in_=ps[:, 0:hw_])

        # Q slice 0 before V: the first attention score matmuls (and with
        # them the scalar exp chain, the global long pole) only need K+Q;
        # V is first consumed a few chunks later by the ctx matmuls.
        qx = []
        qx_bufs = 2 if cfg.LEAN else 4
        x_q = xin.tile([128, cfg.DC, cfg.NSW], BF16, tag="qx",
                       bufs=qx_bufs, name="x_q")
        nc.sync.dma_start(out=x_q, in_=qT[0])
        qx.append(x_q)
        Job(xw_slice(0, qx[0], wq_sb, qhT_sb)).finish()
        # V chunk DMAs go ahead of the remaining (filler-consumed) q slices
        # on the sync queue so vh is ready for the first ctx matmuls.
        for kb in range(min(4, cfg.KB_MAX)):
            Job(v_chunk(kb)).finish()
        for ns in range(1, cfg.NS if not cfg.LEAN else 1):
            x_q = xin.tile([128, cfg.DC, cfg.NSW], BF16, tag="qx",
                           bufs=qx_bufs, name="x_q")
            nc.sync.dma_start(out=x_q, in_=qT[ns])
            qx.append(x_q)

        # filler jobs, keyed by the pair-0 row-range they must finish in:
        # everything keyed r is consumed by row-range r+1 (or later)
        fill_r = {0: [], 1: [], 2: []}
        if not cfg.LEAN:
            for kb in range(4, cfg.KB_MAX):
                fill_r[0].append(Job(v_chunk(kb)))
            for ns in range(1, cfg.NS_K):
                kx = xin.tile([128, cfg.DC, cfg.NSW], BF16, tag="x_ns",
                              name="kx")
                nc.sync.dma_start(out=kx, in_=kT[ns])
                fill_r[0].append(Job(xw_slice(ns, kx, wk_sb, khT_sb)))
            for ns in range(1, cfg.NS):
                fill_r[min(ns - 1, 2)].append(
                    Job(xw_slice(ns, qx[ns], wq_sb, qhT_sb)))
        else:
            for kb in range(4, cfg.KB_MAX):
                Job(v_chunk(kb)).finish()
            qk_proj(kT, wk_sb, khT_sb, ns_count=cfg.NS_K, ns_start=1)
            for ns in range(1, cfg.NS):
                x_q = xin.tile([128, cfg.DC, cfg.NSW], BF16, tag="qx",
                               bufs=qx_bufs, name="x_q")
                nc.sync.dma_start(out=x_q, in_=qT[ns])
                qx.append(x_q)
                Job(xw_slice(ns, x_q, wq_sb, qhT_sb)).finish()

        # P3 constants: emitted on the sync queue AFTER P1's input stream so
        # they don't compete for HBM bandwidth before the first matmul; they
        # transfer during P2 and are ready long before P3 needs them.
        nc.sync.dma_start(out=wo_sb, in_=woT)
        nc.sync.dma_start(out=res_sb, in_=resid)
        nc.sync.dma_start(out=g_row, in_=gamma)
        nc.sync.dma_start(out=b_row, in_=beta)
        nc.gpsimd.partition_broadcast(gamma_bc, g_row)
        nc.gpsimd.partition_broadcast(beta_bc, b_row)

        # ---- P2: attention; per-pair A2A overlaps the next pair -----------
        # Both heads' scores go into ONE 2-bank psum tile (cols h2*RNG+...)
        # so a single scalar activation computes exp for both heads.
        def ctx_mm(pair, r, kb, h2, ctx_ps, probs, nch):
            f0 = max(0, kb * 128 - r * RNG)
            h = 2 * pair + h2
            nc.tensor.matmul(
                ctx_ps[:, f0:],
                vh_sb[:, kb, h * (dh + 1):(h + 1) * (dh + 1)],
                probs[:, h2 * RNG + f0:h2 * RNG + RNG],
                start=kb == 0, stop=kb == nch - 1)

        for pair in range(cfg.PAIRS):
            for r in range(cfg.NR):
                nch = min(((r + 1) * RNG) // 128, cfg.KB_MAX)
                jobs = fill_r.pop(r, []) if pair == 0 else []
                nfill = sum(len(j.items) - j.idx for j in jobs)
                per_kb = -(-nfill // nch) if nfill else 0
                ctx_ps = [psum.tile([dh + 1, RNG], F32, tag="ctx",
                                    bufs=4, name=f"ctx_ps{h2}")
                          for h2 in range(2)]
                pend = []  # pending probs tiles awaiting their ctx matmul
                for kb in range(nch):
                    f0 = max(0, kb * 128 - r * RNG)
                    diag = f0 > 0 or kb * 128 == r * RNG
                    sc = psum.tile([128, 2 * RNG], F32, tag="sc", bufs=2,
                                   name="sc")
                    probs = att.tile([128, 2 * RNG], BF16, tag="pr",
                                     bufs=3 if cfg.LEAN else 4, name="probs")
                    # software pipeline (depth 3): ctx of kb-3 interleaves
                    # between the score matmuls of kb so the PE never waits
                    # on the tri-add + merged exp of recent chunks.
                    for h2 in range(2):
                        lo, hi = 64 * h2, 64 * h2 + 64
                        nc.tensor.matmul(
                            sc[:, h2 * RNG + f0:(h2 + 1) * RNG],
                            khT_sb[lo:hi, pair, kb * 128:(kb + 1) * 128],
                            qhT_sb[lo:hi, pair, r * RNG + f0:(r + 1) * RNG],
                            start=True, stop=True)
                        if len(pend) == 3:
                            ctx_mm(pair, r, kb - 3, h2, ctx_ps[h2],
                                   pend[0], nch)
                    if len(pend) == 3:
                        pend.pop(0)
                    if diag:
                        # causal boundary: bias the diagonal band before exp
                        for h2 in range(2):
                            band = slice(h2 * RNG + f0, h2 * RNG + f0 + 128)
                            nc.vector.tensor_add(sc[:, band], sc[:, band],
                                                 tri)
                    nc.scalar.activation(
                        out=probs, in_=sc,
                        func=mybir.ActivationFunctionType.Exp,
                        bias=pb_sb[:, kb:kb + 1],
                        scale=1.0 / math.sqrt(dh))
                    todo = per_kb
                    for j in jobs:
                        if todo <= 0 or j.done():
                            continue
                        take = min(todo, len(j.items) - j.idx)
                        j.emit(take)
                        todo -= take
                    pend.append(probs)
                for j in jobs:
                    j.finish()
                for i, pr_t in enumerate(pend):
                    for h2 in range(2):
                        ctx_mm(pair, r, nch - len(pend) + i, h2,
                               ctx_ps[h2], pr_t, nch)

                # epilogue: divide rows 0..dh-1 by row dh (the prob sum):
                # bounce the denom row to SBUF, fast-approx reciprocal
                # (the DVE RECIPROCAL op costs a flat ~3.3us!), partition
                # broadcast, then multiply straight out of PSUM.
                stage = att.tile([128, RNG], BF16, tag="stage",
                                 bufs=2 if cfg.LEAN else 4)
                sb = 1 if cfg.LEAN else 2
                for h2 in range(2):
                    den = small.tile([1, RNG], F32, tag=f"den{h2}",
                                     name=f"den{h2}", bufs=sb)
                    nc.vector.tensor_copy(out=den, in_=ctx_ps[h2][dh:dh + 1])
                    rec = small.tile([1, RNG], F32, tag=f"rec{h2}",
                                     name=f"rec{h2}", bufs=sb)
                    nc.vector.reciprocal_approx_fast(rec, den)
                    rbc = small.tile([64, RNG], F32, tag=f"rbc{h2}",
                                     name=f"rbc{h2}", bufs=sb)
                    nc.gpsimd.partition_broadcast(rbc, rec)
                    nc.vector.tensor_mul(
                        stage[64 * h2:64 * h2 + 64, :],
                        ctx_ps[h2][0:dh, :], rbc)
                # stage rows r*RNG+[0,RNG) as two A2A slots of RSL rows
                for j in range(2):
                    nc.sync.dma_start(
                        out=a2a_in[pair][2 * r + j, :, :],
                        in_=stage[:, j * RSL:(j + 1) * RSL])
            nc.gpsimd.collective_compute(
                "AllToAll", mybir.AluOpType.bypass,
                replica_groups=[list(range(cfg.NC))],
                ins=[a2a_in[pair][:]], outs=[a2a_out[pair][:]])

        # Scheduler fence (no runtime syncs): without it the scheduler hoists
        # the cc0-gated fetch DMAs ahead of pair-1's staging DMAs on the sync
        # queue, stalling the whole queue until cc0 completes.
        tc.no_sync_barrier()

        # fetch gathered ctx chunks: ccb[(pair, sender)] = sender's 2 heads
        # (128 dims) of pair `pair`, for my RQ rows (RSL per batch half).
        ccb = {}
        for pair in range(cfg.PAIRS):
            for s in range(cfg.NC):
                t_ccb = ctxf.tile([128, RSL], BF16, name=f"ccb_{pair}_{s}",
                                  tag=f"ccb_{pair}_{s}")
                nc.sync.dma_start(out=t_ccb, in_=a2a_out[pair][s, :, :])
                ccb[(pair, s)] = t_ccb

        # ---- P3: Wo + residual + LayerNorm ---------------------------------
        # row-tile t covers my rows [t*128,(t+1)*128): batch b = t//2,
        # in-slot column range (t%2)*128. Sender 4b+s holds head chunk
        # (pair, s) for that batch. Round A accumulates every pair-0 chunk
        # for ALL row-tiles into SBUF partials while the pair-1 collective
        # is still in flight; round B adds the pair-1 chunks.
        def wo_round(t, pair, pso):
            b = t // 2
            col = slice((t % 2) * 128, (t % 2) * 128 + 128)
            for s in range(cfg.G):
                cc = ccb[(pair, 4 * b + s)][:, col]
                # global output dim chunk for (sender s, pair):
                oc = s * cfg.D4C + pair
                for nsl in range(cfg.WON):
                    nc.tensor.matmul(
                        pso[nsl], cc,
                        wo_sb[:, oc, nsl * cfg.WONW:(nsl + 1) * cfg.WONW],
                        start=s == 0, stop=s == cfg.G - 1)

        partA = []
        for t in range(RQ // 128):
            pso = [psum.tile([128, cfg.WONW], F32, tag="ctx",
                             bufs=4, name=f"psoA{nsl}")
                   for nsl in range(cfg.WON)]
            wo_round(t, 0, pso)
            pa = lnp.tile([128, D], BF16, tag="partA", bufs=4)
            for nsl in range(cfg.WON):
                sl = slice(nsl * cfg.WONW, (nsl + 1) * cfg.WONW)
                nc.vector.tensor_add(pa[:, sl], pso[nsl], res_sb[:, t, sl])
            partA.append(pa)

        for t in range(RQ // 128):
            pso = [psum.tile([128, cfg.WONW], F32, tag="ctx",
                             bufs=4, name=f"psoB{nsl}")
                   for nsl in range(cfg.WON)]
            wo_round(t, 1, pso)
            x = lnp.tile([128, D], F32, tag="x")
            for nsl in range(cfg.WON):
                sl = slice(nsl * cfg.WONW, (nsl + 1) * cfg.WONW)
                nc.vector.tensor_add(x[:, sl], pso[nsl], partA[t][:, sl])
            fmax = math.gcd(nc.vector.BN_STATS_FMAX, D)
            nsub = D // fmax
            stats = lnp.tile([128, nsub, nc.vector.BN_STATS_DIM], F32,
                             tag="stats")
            for sg in range(nsub):
                nc.vector.bn_stats(
                    out=stats[:, sg, :],
                    in_=x.rearrange("p (a b) -> p a b", a=nsub)[:, sg, :])
            mv = lnp.tile([128, nc.vector.BN_AGGR_DIM], F32, tag="mv")
            nc.vector.bn_aggr(out=mv, in_=stats)
            sd = lnp.tile([128, 1], F32, tag="sd")
            nc.scalar.activation(out=sd, in_=mv[:, 1:2],
                                 func=mybir.ActivationFunctionType.Sqrt,
                                 bias=eps_sb, scale=1.0)
            rstd = lnp.tile([128, 1], F32, tag="rstd")
            nc.vector.reciprocal_approx_fast(rstd, sd)
            out_sb = lnp.tile([128, D], BF16, tag="out_sb")
            if cfg.G1 and cfg.B0:
                # gamma==1, beta==0: normalize straight into the output
                nc.vector.tensor_scalar(
                    out=out_sb, in0=x, scalar1=mv[:, 0:1], scalar2=rstd,
                    op0=mybir.AluOpType.subtract, op1=mybir.AluOpType.mult)
            else:
                y = lnp.tile([128, D], BF16, tag="y")
                nc.vector.tensor_scalar(
                    out=y, in0=x, scalar1=mv[:, 0:1], scalar2=rstd,
                    op0=mybir.AluOpType.subtract, op1=mybir.AluOpType.mult)
                if cfg.B0:
                    nc.vector.tensor_mul(out_sb, y, gamma_bc)
                elif cfg.G1:
                    nc.vector.tensor_add(out_sb, y, beta_bc)
                else:
                    yg = lnp.tile([128, D], BF16, tag="yg")
                    nc.vector.tensor_mul(yg, y, gamma_bc)
                    nc.vector.tensor_add(out_sb, yg, beta_bc)
            nc.sync.dma_start(out=out_shard[t * 128:(t + 1) * 128, :],
                              in_=out_sb)

    nc.compile()
    return nc


def _tile_x(xT, ns_count, nsw, dc=8):
    """[D, S'] -> [ns, 128, dc, nsw] so each n-slice DMA is contiguous."""
    d, s = xT.shape
    cols = ns_count * nsw
    out = xT[:, :cols].reshape(dc, 128, ns_count, nsw)
    return np.ascontiguousarray(out.transpose(2, 1, 0, 3))


def _tile_w(wT):
    """[D, O] -> [128, dc, O] so the weight DMA is contiguous."""
    d, o = wT.shape
    return np.ascontiguousarray(wT.reshape(d // 128, 128, o).transpose(1, 0, 2))


def make_in_maps(cfg: Cfg, q, k, v, Wq, Wk, Wv, Wo, gamma, beta, sen_len):
    """Host-side sharding: slice/transpose/cast/tile per core."""
    bf = ml_dtypes.bfloat16
    in_maps = []
    woT_full = _tile_w(Wo.T.astype(bf))
    pos = np.arange(cfg.S)
    per_batch = {}
    for b in range(cfg.B):
        per_batch[b] = (
            _tile_x(q[b].T.astype(bf), cfg.NS, cfg.NSW),
            _tile_x(k[b].T.astype(bf), cfg.NS_K, cfg.NSW),
            _tile_x(v[b].T.astype(bf), cfg.KB_MAX, 128),
            np.ascontiguousarray(
                np.where(pos < int(sen_len[b]), 0.0, NEG_INF)
                .astype(np.float32).reshape(cfg.KCH, 128).T),
        )
    for c in range(cfg.NC):
        g = c // cfg.G
        l = c % cfg.G
        hs = slice(l * cfg.D4, (l + 1) * cfg.D4)
        rows = slice(c * cfg.RSL, (c + 1) * cfg.RSL)
        qTb, kTb, vTb, pb = per_batch[g]
        res = np.concatenate([q[b, rows, :] for b in range(cfg.B)], axis=0)
        res = res.astype(np.float32).reshape(cfg.G, 128, cfg.D)
        in_maps.append({
            "qT": qTb, "kT": kTb, "vT": vTb,
            "wqT": _tile_w(Wq[hs, :].T.astype(bf)),
            "wkT": _tile_w(Wk[hs, :].T.astype(bf)),
            "wvT": _tile_w(Wv[hs, :].T.astype(bf)),
            "woT": woT_full,
            "resid": np.ascontiguousarray(res.transpose(1, 0, 2)),
            "pad_bias": pb,
            "gamma": gamma.reshape(1, cfg.D).astype(bf),
            "beta": beta.reshape(1, cfg.D).astype(bf),
        })
    return in_maps


def assemble_output(cfg: Cfg, results):
    out = np.empty((cfg.B, cfg.S, cfg.D), np.float32)
    for c in range(cfg.NC):
        rows = slice(c * cfg.RSL, (c + 1) * cfg.RSL)
        for b in range(cfg.B):
            out[b, rows, :] = results[c]["out_shard"][
                b * cfg.RSL:(b + 1) * cfg.RSL].astype(np.float32)
    return out


_PROGRAM_CACHE = {}


def _get_program(cfg: Cfg):
    key = (cfg.B, cfg.S, cfg.D, cfg.H, cfg.dh, cfg.KB_MAX, cfg.G1, cfg.B0)
    if key not in _PROGRAM_CACHE:
        _PROGRAM_CACHE[key] = build_program(cfg)
    return _PROGRAM_CACHE[key]


def run(cfg: Cfg, inputs: dict, trace: bool = False):
    cfg.G1 = bool(np.all(np.asarray(inputs["gamma"]) == 1.0))
    cfg.B0 = bool(np.all(np.asarray(inputs["beta"]) == 0.0))
    nc = _get_program(cfg)
    in_maps = make_in_maps(cfg, **inputs)
    res = run_bass_kernel_spmd(nc, in_maps, core_ids=list(range(cfg.NC)),
                               trace=trace)
    return assemble_output(cfg, res.results), res


def kernel(**inputs) -> np.ndarray:
    kmax = int(np.max(inputs["sen_len"]))
    cfg = Cfg(B=2, S=2048, D=1024, H=16, dh=64, kmax=kmax)
    out, _ = run(cfg, inputs)
    return out



# revision 41
# speedup vs baseline: 1.4524x; 1.4524x over previous
"""Multi-head attention (QKV projections + causal/padded softmax attention +
output projection + residual + LayerNorm) as a Bass/Tile kernel on 8 Trainium2
cores — NO collectives.

Sharding: rows (sequence) are sharded across cores; every core computes ALL 16
heads for its own 512 rows end-to-end, so no cross-core communication is ever
needed.  Core c handles batch b = c//4 and the four 128-row tiles
t_j = 4*j + (c%4), j = 0..3 (interleaved so early/late causal tiles spread
evenly).  The price is that each 4-core batch group re-computes the batch's
K/V projections (up to kmax keys) redundantly; that costs ~34us of PE but
saves the ~100us collective chain (CC barrier + 2 AllToAlls) the head-sharded
variant pays.

SPMD trick for the causal mask: the program is identical on all cores, but the
position of the causal diagonal inside each row-tile's key loop is
core-dependent.  All key/row masking is therefore driven by per-core INPUT
constants applied along hardware-broadcast axes only:
  - bias_sb[key, (j, kb)]: per-key exp bias = 0 (valid) / -1e9 (key padded or
    chunk entirely above the diagonal); consumed as the scalar-activation
    per-partition bias of the fused exp, so masking is free.
  - alpha[(j, kb)]: per-core scalar that multiplies a static upper-triangle
    0/1 constant (TRIrep8, replicated per head) added onto the scores psum by
    ONE scalar_tensor_tensor per (row-tile, chunk) position where ANY core
    could have its diagonal: sc += alpha * TRI.  alpha = -1e9 exactly on this
    core's diagonal chunk, 0 elsewhere.
Scores live as sc[key, row] (key on partitions) so the pad mask is a
per-partition column; V is augmented with a ones column so row 64 of the ctx
psum accumulates the softmax denominators (no extra matmul).

Matmul cost on TRN2 = moving-column count (contraction/output width free), so
all operand layouts are chosen to minimize total moving columns:
Q 32.8k + K 8*8*kpad + V same + scores/ctx 16*sum(cap_j)*128 each + Wo 32.8k
cycles at 2.4 GHz (p-state held by a dense back-to-back PE stream).
"""

import math
import os
from contextlib import ExitStack

_KVAR = os.environ.get("KVAR", "")  # temporary debug probe selector

import numpy as np
import ml_dtypes

import concourse.mybir as mybir
import concourse.tile as tile
from concourse import bacc
from concourse.bass_utils import run_bass_kernel_spmd

BF16 = mybir.dt.bfloat16
F32 = mybir.dt.float32

NEG_INF = -1e9
LN_EPS = 1e-6


class Cfg:
    def __init__(self, B=2, S=2048, D=1024, H=16, dh=64, kmax=None):
        self.B, self.S, self.D, self.H, self.dh = B, S, D, H, dh
        self.kmax = S if kmax is None else max(1, min(int(kmax), S))
        self.NC = 8                       # cores
        self.G = 4                        # cores per batch group
        self.RPC = S // self.G            # rows per core (512)
        self.NT = self.RPC // 128         # row-tiles per core (4)
        self.DC = D // 128                # contraction chunks (8)
        self.NP = H // 2                  # head pairs (8)
        self.KB = -(-self.kmax // 128)    # key chunks actually needed
        self.KPAD = self.KB * 128
        # slot j covers row tile 4*j+q (q = core quarter); the static chunk
        # cap must cover the deepest core (q=3)
        self.caps = [min(4 * j + 4, self.KB) for j in range(self.NT)]
        # (j, kb) positions where ANY core's causal diagonal can fall
        self.POS = [(j, kb) for j in range(self.NT)
                    for kb in range(self.caps[j])
                    if 4 * j <= kb <= 4 * j + 3]
        # runtime-detected LN specializations
        self.G1 = False
        self.B0 = False


def build_program(cfg: Cfg):
    nc = bacc.Bacc("TRN2", target_bir_lowering=False, debug=False,
                   num_devices=cfg.NC)

    D, dh = cfg.D, cfg.dh
    KB, KPAD, RPC, NT = cfg.KB, cfg.KPAD, cfg.RPC, cfg.NT

    xq = nc.dram_tensor("xq", [128, cfg.DC, RPC], BF16,
                        kind="ExternalInput").ap()
    xk = nc.dram_tensor("xk", [128, cfg.DC, KPAD], BF16,
                        kind="ExternalInput").ap()
    xv = nc.dram_tensor("xv", [KB, 128, cfg.DC, 128], BF16,
                        kind="ExternalInput").ap()
    wqT = nc.dram_tensor("wqT", [128, cfg.DC, D], BF16,
                         kind="ExternalInput").ap()
    wkT = nc.dram_tensor("wkT", [128, cfg.DC, D], BF16,
                         kind="ExternalInput").ap()
    wvT = nc.dram_tensor("wvT", [128, cfg.DC, D], BF16,
                         kind="ExternalInput").ap()
    woT = nc.dram_tensor("woT", [128, cfg.DC, D], BF16,
                         kind="ExternalInput").ap()
    bias_in = nc.dram_tensor("bias_in", [128, NT * KB], F32,
                             kind="ExternalInput").ap()
    alpha_in = nc.dram_tensor("alpha_in", [128, max(1, len(cfg.POS))], F32,
                              kind="ExternalInput").ap()
    resid = nc.dram_tensor("resid", [128, NT, D], F32,
                           kind="ExternalInput").ap()
    gamma = nc.dram_tensor("gamma", [1, D], BF16, kind="ExternalInput").ap()
    beta = nc.dram_tensor("beta", [1, D], BF16, kind="ExternalInput").ap()
    out_shard = nc.dram_tensor("out_shard", [RPC, D], BF16,
                               kind="ExternalOutput").ap()

    with tile.TileContext(nc) as tc, ExitStack() as ctx:
        consts = ctx.enter_context(tc.tile_pool(name="consts", bufs=1))
        xin = ctx.enter_context(tc.tile_pool(name="xin", bufs=1))
        proj = ctx.enter_context(tc.tile_pool(name="proj", bufs=1))
        att = ctx.enter_context(tc.tile_pool(name="att", bufs=2))
        small = ctx.enter_context(tc.tile_pool(name="small", bufs=2))
        lnp = ctx.enter_context(tc.tile_pool(name="lnp", bufs=2))
        psum = ctx.enter_context(
            tc.tile_pool(name="psum", bufs=1, space="PSUM"))

        # ---- constants -----------------------------------------------------
        # weights ride the scalar engine's DMA queue (parallel to the sync
        # queue that streams activations); per-dc chunks so the first
        # accumulation chain can start after 1/8 of the weight transfer.
        # wk and wo share one slot (tag ring): wk is dead after the K
        # projection, long before Wo is needed, so wo's DMA just waits.
        wk_sb = consts.tile([128, cfg.DC, D], BF16, tag="wko", bufs=1,
                            name="wk_sb")
        wq_sb = consts.tile([128, cfg.DC, D], BF16)
        wv_sb = consts.tile([128, cfg.DC, D], BF16)
        for dc in range(cfg.DC):
            nc.scalar.dma_start(out=wk_sb[:, dc, :], in_=wkT[:, dc, :])
        for dc in range(cfg.DC):
            nc.scalar.dma_start(out=wq_sb[:, dc, :], in_=wqT[:, dc, :])
        for dc in range(cfg.DC):
            nc.scalar.dma_start(out=wv_sb[:, dc, :], in_=wvT[:, dc, :])

        bias_sb = consts.tile([128, NT * KB], F32)
        alpha_sb = consts.tile([128, max(1, len(cfg.POS))], F32)
        nc.sync.dma_start(out=bias_sb, in_=bias_in)
        nc.sync.dma_start(out=alpha_sb, in_=alpha_in)

        # TRIrep8[p, h, f] = 1.0 where row f < key p else 0 (upper triangle),
        # replicated across the 8 head slots of one score-group tile.
        trirep = consts.tile([128, 8, 128], F32)
        nc.vector.memset(trirep, 0.0)
        for s in range(8):
            nc.gpsimd.affine_select(
                out=trirep[:, s, :], in_=trirep[:, s, :], pattern=[[1, 128]],
                base=0, channel_multiplier=-1,
                compare_op=mybir.AluOpType.is_ge, fill=1.0)

        # P3 constants (loaded later on the sync queue, behind the inputs)
        gamma_bc = beta_bc = None
        if not (cfg.G1 and cfg.B0):
            g_row = consts.tile([1, D], BF16)
            b_row = consts.tile([1, D], BF16)
            gamma_bc = consts.tile([128, D], BF16)
            beta_bc = consts.tile([128, D], BF16)
        eps_sb = consts.tile([128, 1], F32)
        nc.vector.memset(eps_sb, LN_EPS)
        res_sb = consts.tile([128, NT, D], F32)

        # ---- P1: projections ----------------------------------------------
        khT_sb = proj.tile([128, cfg.NP, KPAD], BF16)
        qhT_sb = proj.tile([128, cfg.NP, RPC], BF16)
        vh_sb = proj.tile([128, KB, cfg.H, dh + 1], BF16)
        nc.gpsimd.memset(vh_sb[:, :, :, dh:dh + 1], 1.0)

        xk_sb = xin.tile([128, cfg.DC, KPAD], BF16)
        # first half-slice DMA'd alone so the first matmul starts early
        nc.sync.dma_start(out=xk_sb[:, :, 0:256], in_=xk[:, :, 0:256])
        nc.sync.dma_start(out=xk_sb[:, :, 256:KPAD], in_=xk[:, :, 256:KPAD])
        xq_sb = xin.tile([128, cfg.DC, RPC], BF16)
        nc.sync.dma_start(out=xq_sb, in_=xq)
        xv_sb = []
        for kb in range(KB):
            t = xin.tile([128, cfg.DC, 128], BF16, tag="xv", bufs=KB,
                         name=f"xv{kb}")
            nc.sync.dma_start(out=t, in_=xv[kb])
            xv_sb.append(t)

        # remaining P3 constants transfer during P1/P2
        nc.sync.dma_start(out=res_sb, in_=resid)
        if gamma_bc is not None:
            nc.sync.dma_start(out=g_row, in_=gamma)
            nc.sync.dma_start(out=b_row, in_=beta)
            nc.gpsimd.partition_broadcast(gamma_bc, g_row)
            nc.gpsimd.partition_broadcast(beta_bc, b_row)

        copy_flip = [0]

        def pcopy(out_ap, in_ap):
            # alternate psum->sbuf copies between DVE and the scalar engine
            eng = nc.vector if copy_flip[0] % 2 == 0 else nc.scalar
            copy_flip[0] += 1
            if eng is nc.vector:
                eng.tensor_copy(out=out_ap, in_=in_ap)
            else:
                eng.copy(out=out_ap, in_=in_ap)

        def xw_proj(x_sb, w_sb, out_sb, width):
            # out[128 dims (pair), cols] = sum_dc w[:, dc, pair]^T? -- PE:
            # lhsT = w slice [128, 128], rhs = x [128, cols]
            for p in range(cfg.NP):
                for c0 in range(0, width, 512):
                    cw = min(512, width - c0)
                    ps = psum.tile([128, cw], F32, tag="ctx", bufs=4,
                                   name="ps_proj")
                    for dc in range(cfg.DC):
                        nc.tensor.matmul(
                            ps, w_sb[:, dc, p * 128:(p + 1) * 128],
                            x_sb[:, dc, c0:c0 + cw],
                            start=dc == 0, stop=dc == cfg.DC - 1)
                    pcopy(out_sb[:, p, c0:c0 + cw], ps)

        # K first (scores need it first), then Q, then V
        xw_proj(xk_sb, wk_sb, khT_sb, KPAD)
        xw_proj(xq_sb, wq_sb, qhT_sb, RPC)

        for kb in range(KB):
            for half in range(2):
                ps = psum.tile([128, 512], F32, tag="ctx", bufs=4,
                               name="ps_v")
                for dc in range(cfg.DC):
                    nc.tensor.matmul(
                        ps, xv_sb[kb][:, dc, :],
                        wv_sb[:, dc, half * 512:half * 512 + 512],
                        start=dc == 0, stop=dc == cfg.DC - 1)
                pcopy(
                    vh_sb[:, kb, 8 * half:8 * half + 8, 0:dh],
                    ps.rearrange("p (h e) -> p h e", e=dh))

        # wo reuses wk's SBUF slot (wk is dead now); DMA overlaps attention
        wo_sb = consts.tile([128, cfg.DC, D], BF16, tag="wko", bufs=1,
                            name="wo_sb")
        for dc in range(cfg.DC):
            nc.scalar.dma_start(out=wo_sb[:, dc, :], in_=woT[:, dc, :])

        # ---- P2: attention -------------------------------------------------
        ctxT_sb = proj.tile([128, cfg.NP, RPC], BF16)
        pos_idx = {pos: i for i, pos in enumerate(cfg.POS)}

        if _KVAR in ("nop2", "nop3", "nodiv", "noexp", "noctx", "noexp2"):
            for rt in range(NT):
                o = lnp.tile([128, D], BF16, tag="out_sb")
                nc.vector.memset(o, 0.0)
                nc.sync.dma_start(
                    out=out_shard[rt * 128:(rt + 1) * 128, :], in_=o)

        n_real = NT
        if _KVAR == "nop2":
            n_real = 0
        elif _KVAR.startswith("j") and _KVAR.split("-")[0][1:].isdigit():
            n_real = int(_KVAR.split("-")[0][1:])
            for j in range(n_real, NT):
                nc.vector.memset(ctxT_sb[:, :, j * 128:(j + 1) * 128], 0.0)

        for j in range(n_real):
            cap = cfg.caps[j]
            # 4 ctx psum banks for the slot: bank 2g+bb holds heads
            # (8g+4bb)..(8g+4bb+3), each 128 cols; row 64 = denominators
            ctx_b = [psum.tile([dh + 1, 512], F32, tag="ctx", bufs=4,
                               name=f"ctxps{b}") for b in
                     range(4 if _KVAR != "noexp2" and "fkc" not in _KVAR
                           else 0)]
            pend = []  # (kb, g, probs) awaiting their ctx matmuls
            for kb in range(cap):
                for g in range(2):
                    sc = psum.tile([128, 1024], F32, tag="sc", bufs=2,
                                   name="sc")
                    # HW constraint: matmuls whose stationary sits at
                    # different partition bases (PE row-tile 0 vs 64) must
                    # NOT write the same PSUM bank.  Even heads (base 0) go
                    # to bank 0 (cols 0-511), odd heads (base 64) to bank 1
                    # (cols 512-1023); one accumulation group per bank.
                    for hh in range(8):
                        h = 8 * g + hh
                        p, h2 = h // 2, h % 2
                        lo = 64 * h2
                        col = h2 * 512 + (hh // 2) * 128
                        nc.tensor.matmul(
                            sc[:, col:col + 128],
                            khT_sb[lo:lo + 64, p, kb * 128:(kb + 1) * 128],
                            qhT_sb[lo:lo + 64, p, j * 128:(j + 1) * 128],
                            start=hh < 2, stop=hh >= 6)
                    if _KVAR in ("noexp", "noexp2"):
                        continue
                    if (j, kb) in pos_idx and "nomsk" not in _KVAR:
                        i = pos_idx[(j, kb)]
                        msk = att.tile([128, 1024], F32, tag="msk", bufs=2,
                                       name="msk")
                        nc.vector.tensor_scalar_mul(
                            msk, trirep, alpha_sb[:, i:i + 1])
                        nc.vector.tensor_add(sc, sc, msk)
                    probs = att.tile([128, 1024], BF16, tag="pr", bufs=4,
                                     name="probs")
                    if "cpd" in _KVAR:
                        nc.vector.tensor_copy(out=probs, in_=sc)
                    else:
                        nc.scalar.activation(
                            out=probs, in_=sc,
                            func=mybir.ActivationFunctionType.Exp,
                            bias=bias_sb[:, j * KB + kb:j * KB + kb + 1],
                            scale=1.0 / math.sqrt(dh))
                    if "fkc" in _KVAR:
                        nc.vector.tensor_copy(
                            out=ctxT_sb[:, :, j * 128:(j + 1) * 128],
                            in_=probs)
                        continue
                    if _KVAR == "noctx":
                        continue
                    if len(pend) == 2:
                        okb, og, opr = pend.pop(0)
                        for hh in range(8):
                            h = 8 * og + hh
                            pcol = (hh % 2) * 512 + (hh // 2) * 128
                            nc.tensor.matmul(
                                ctx_b[2 * og + hh // 4]
                                [:, (hh % 4) * 128:(hh % 4) * 128 + 128],
                                vh_sb[:, okb, h, :],
                                opr[:, pcol:pcol + 128],
                                start=okb == 0 and hh % 4 == 0,
                                stop=okb == cap - 1 and hh % 4 == 3)
                    pend.append((kb, g, probs))
            for okb, og, opr in pend:
                for hh in range(8):
                    h = 8 * og + hh
                    pcol = (hh % 2) * 512 + (hh // 2) * 128
                    nc.tensor.matmul(
                        ctx_b[2 * og + hh // 4]
                        [:, (hh % 4) * 128:(hh % 4) * 128 + 128],
                        vh_sb[:, okb, h, :],
                        opr[:, pcol:pcol + 128],
                        start=okb == 0 and hh % 4 == 0,
                        stop=okb == cap - 1 and hh % 4 == 3)

            # divide by the denominators (psum row 64) and stage ctxT for Wo
            for b in range(4 if _KVAR not in ("nodiv", "noexp", "noctx",
                                              "noexp2")
                           and "fkc" not in _KVAR else 0):
                rbc = small.tile([64, 512], F32, tag=f"rbc{b % 2}", bufs=2,
                                 name=f"rbc{b % 2}")
                if "nde" in _KVAR:
                    nc.vector.memset(rbc, 1.0)
                else:
                    den = small.tile([1, 512], F32, tag=f"den{b % 2}",
                                     bufs=2, name=f"den{b % 2}")
                    nc.vector.tensor_copy(out=den,
                                          in_=ctx_b[b][dh:dh + 1, :])
                    rec = small.tile([1, 512], F32, tag=f"rec{b % 2}",
                                     bufs=2, name=f"rec{b % 2}")
                    nc.vector.reciprocal_approx_fast(rec, den)
                    nc.gpsimd.partition_broadcast(rbc, rec)
                # bank b holds heads h0..h0+3 (pairs p0, p0+1); head parity
                # alternates partition halves of ctxT_sb
                h0 = 8 * (b // 2) + 4 * (b % 2)
                p0 = h0 // 2
                if "div2" in _KVAR:
                    for hh in range(4):
                        h = h0 + hh
                        nc.vector.tensor_mul(
                            ctxT_sb[64 * (h % 2):64 * (h % 2) + 64, h // 2,
                                    j * 128:(j + 1) * 128],
                            ctx_b[b][0:dh, hh * 128:(hh + 1) * 128],
                            rbc[:, hh * 128:(hh + 1) * 128])
                else:
                    for par in range(2):
                        nc.vector.tensor_mul(
                            ctxT_sb[64 * par:64 * par + 64, p0:p0 + 2,
                                    j * 128:(j + 1) * 128],
                            ctx_b[b][0:dh, :]
                            .rearrange("p (a c) -> p a c", a=4)[:, par::2, :],
                            rbc.rearrange("p (a c) -> p a c", a=4)
                            [:, par::2, :])

        # ---- P3: Wo + residual + LayerNorm ---------------------------------
        for rt in range(NT if _KVAR not in ("nop2", "nop3", "nodiv", "noexp")
                        else 0):
            pso = [psum.tile([128, 512], F32, tag="ctx", bufs=4,
                             name=f"pso{ns}") for ns in range(2)]
            for p in range(cfg.NP):
                for ns in range(2):
                    nc.tensor.matmul(
                        pso[ns], ctxT_sb[:, p, rt * 128:(rt + 1) * 128],
                        wo_sb[:, p, ns * 512:ns * 512 + 512],
                        start=p == 0, stop=p == cfg.NP - 1)
            x = lnp.tile([128, D], F32, tag="x")
            for ns in range(2):
                nc.vector.tensor_add(x[:, ns * 512:ns * 512 + 512], pso[ns],
                                     res_sb[:, rt, ns * 512:ns * 512 + 512])
            fmax = math.gcd(nc.vector.BN_STATS_FMAX, D)
            nsub = D // fmax
            stats = lnp.tile([128, nsub, nc.vector.BN_STATS_DIM], F32,
                             tag="stats")
            for sg in range(nsub):
                nc.vector.bn_stats(
                    out=stats[:, sg, :],
                    in_=x.rearrange("p (a b) -> p a b", a=nsub)[:, sg, :])
            mv = lnp.tile([128, nc.vector.BN_AGGR_DIM], F32, tag="mv")
            nc.vector.bn_aggr(out=mv, in_=stats)
            sd = lnp.tile([128, 1], F32, tag="sd")
            nc.scalar.activation(out=sd, in_=mv[:, 1:2],
                                 func=mybir.ActivationFunctionType.Sqrt,
                                 bias=eps_sb, scale=1.0)
            rstd = lnp.tile([128, 1], F32, tag="rstd")
            nc.vector.reciprocal_approx_fast(rstd, sd)
            out_sb = lnp.tile([128, D], BF16, tag="out_sb")
            if cfg.G1 and cfg.B0:
                nc.vector.tensor_scalar(
                    out=out_sb, in0=x, scalar1=mv[:, 0:1], scalar2=rstd,
                    op0=mybir.AluOpType.subtract, op1=mybir.AluOpType.mult)
            else:
                y = lnp.tile([128, D], BF16, tag="y")
                nc.vector.tensor_scalar(
                    out=y, in0=x, scalar1=mv[:, 0:1], scalar2=rstd,
                    op0=mybir.AluOpType.subtract, op1=mybir.AluOpType.mult)
                if cfg.B0:
                    nc.vector.tensor_mul(out_sb, y, gamma_bc)
                elif cfg.G1:
                    nc.vector.tensor_add(out_sb, y, beta_bc)
                else:
                    yg = lnp.tile([128, D], BF16, tag="yg")
                    nc.vector.tensor_mul(yg, y, gamma_bc)
                    nc.vector.tensor_add(out_sb, yg, beta_bc)
            nc.sync.dma_start(out=out_shard[rt * 128:(rt + 1) * 128, :],
                              in_=out_sb)

    nc.compile()
    return nc


def _tile_x(xT, dc=8):
    """[D, C] -> [128, dc, C] so the DMA is contiguous per partition."""
    d, c = xT.shape
    return np.ascontiguousarray(xT.reshape(dc, 128, c).transpose(1, 0, 2))


def _tile_w(wT):
    d, o = wT.shape
    return np.ascontiguousarray(
        wT.reshape(d // 128, 128, o).transpose(1, 0, 2))


def make_in_maps(cfg: Cfg, q, k, v, Wq, Wk, Wv, Wo, gamma, beta, sen_len):
    bf = ml_dtypes.bfloat16
    q = np.asarray(q, np.float32)
    k = np.asarray(k, np.float32)
    v = np.asarray(v, np.float32)
    wq_t = _tile_w(np.asarray(Wq, np.float32).T.astype(bf))
    wk_t = _tile_w(np.asarray(Wk, np.float32).T.astype(bf))
    wv_t = _tile_w(np.asarray(Wv, np.float32).T.astype(bf))
    wo_t = _tile_w(np.asarray(Wo, np.float32).T.astype(bf))
    g_row = np.asarray(gamma, np.float32).reshape(1, cfg.D).astype(bf)
    b_row = np.asarray(beta, np.float32).reshape(1, cfg.D).astype(bf)

    KB, KPAD, NT = cfg.KB, cfg.KPAD, cfg.NT
    per_batch = {}
    for b in range(cfg.B):
        kT = k[b, :KPAD, :].T.astype(bf)          # [D, KPAD]
        vT = v[b, :KPAD, :].T.astype(bf)
        xv = np.ascontiguousarray(
            vT.reshape(cfg.DC, 128, KB, 128).transpose(2, 1, 0, 3))
        per_batch[b] = (_tile_x(kT), xv)

    key_pos = np.arange(KPAD)
    in_maps = []
    for c in range(cfg.NC):
        b, qq = c // cfg.G, c % cfg.G
        tiles = [4 * j + qq for j in range(NT)]
        rows = np.concatenate(
            [np.arange(t * 128, (t + 1) * 128) for t in tiles])
        sl = int(np.asarray(sen_len)[b])

        bias = np.full((128, NT * KB), 0.0, np.float32)
        pad = np.where(key_pos < sl, 0.0, NEG_INF).astype(np.float32)
        for j in range(NT):
            d_j = tiles[j]  # diagonal chunk of this slot
            for kb in range(KB):
                col = pad[kb * 128:(kb + 1) * 128] if kb <= d_j \
                    else np.full(128, NEG_INF, np.float32)
                bias[:, j * KB + kb] = col
        alpha = np.zeros((128, max(1, len(cfg.POS))), np.float32)
        for i, (j, kb) in enumerate(cfg.POS):
            if kb == tiles[j]:
                alpha[:, i] = NEG_INF

        xq_h = _tile_x(q[b][rows, :].T.astype(bf))
        res = np.ascontiguousarray(
            q[b][rows, :].reshape(NT, 128, cfg.D)
            .transpose(1, 0, 2).astype(np.float32))
        kT_t, xv_t = per_batch[b]
        in_maps.append({
            "xq": xq_h, "xk": kT_t, "xv": xv_t,
            "wqT": wq_t, "wkT": wk_t, "wvT": wv_t, "woT": wo_t,
            "bias_in": bias, "alpha_in": alpha, "resid": res,
            "gamma": g_row, "beta": b_row,
        })
    return in_maps


def assemble_output(cfg: Cfg, results):
    out = np.empty((cfg.B, cfg.S, cfg.D), np.float32)
    for c in range(cfg.NC):
        b, qq = c // cfg.G, c % cfg.G
        shard = results[c]["out_shard"].astype(np.float32)
        for j in range(cfg.NT):
            t = 4 * j + qq
            out[b, t * 128:(t + 1) * 128, :] = shard[j * 128:(j + 1) * 128]
    return out


_PROGRAM_CACHE = {}


def _get_program(cfg: Cfg):
    key = (cfg.B, cfg.S, cfg.D, cfg.H, cfg.dh, cfg.KB, cfg.G1, cfg.B0)
    if key not in _PROGRAM_CACHE:
        _PROGRAM_CACHE[key] = build_program(cfg)
    return _PROGRAM_CACHE[key]


def run(cfg: Cfg, inputs: dict, trace: bool = False):
    cfg.G1 = bool(np.all(np.asarray(inputs["gamma"]) == 1.0))
    cfg.B0 = bool(np.all(np.asarray(inputs["beta"]) == 0.0))
    nc = _get_program(cfg)
    in_maps = make_in_maps(cfg, **inputs)
    res = run_bass_kernel_spmd(nc, in_maps, core_ids=list(range(cfg.NC)),
                               trace=trace)
    return assemble_output(cfg, res.results), res


def kernel(**inputs) -> np.ndarray:
    kmax = int(np.max(inputs["sen_len"]))
    cfg = Cfg(B=2, S=2048, D=1024, H=16, dh=64, kmax=kmax)
    out, _ = run(cfg, inputs)
    return out


# revision 52
# speedup vs baseline: 1.4588x; 1.0044x over previous
"""Multi-head attention (QKV projections + causal/padded softmax attention +
output projection + residual + LayerNorm) as a Bass/Tile kernel on 8 Trainium2
cores — NO collectives.

Sharding: rows (sequence) are sharded across cores; every core computes ALL 16
heads for its own 512 rows end-to-end, so no cross-core communication is ever
needed.  Core c handles batch b = c//4 and the four 128-row tiles
t_j = 4*j + (c%4), j = 0..3 (interleaved so early/late causal tiles spread
evenly).  The price is that each 4-core batch group re-computes the batch's
K/V projections (up to kmax keys) redundantly; that costs ~34us of PE but
saves the ~100us collective chain (CC barrier + 2 AllToAlls) the head-sharded
variant pays.

SPMD trick for the causal mask: the program is identical on all cores, but the
position of the causal diagonal inside each row-tile's key loop is
core-dependent.  All key/row masking is therefore driven by per-core INPUT
constants applied along hardware-broadcast axes only:
  - bias_sb[key, (j, kb)]: per-key exp bias = 0 (valid) / -1e9 (key padded or
    chunk entirely above the diagonal); consumed as the scalar-activation
    per-partition bias of the fused exp, so masking is free.
  - alpha[(j, kb)]: per-core scalar that multiplies a static upper-triangle
    0/1 constant (TRIrep8, replicated per head) added onto the scores psum by
    ONE scalar_tensor_tensor per (row-tile, chunk) position where ANY core
    could have its diagonal: sc += alpha * TRI.  alpha = -1e9 exactly on this
    core's diagonal chunk, 0 elsewhere.
Scores live as sc[key, row] (key on partitions) so the pad mask is a
per-partition column; V is augmented with a ones column so row 64 of the ctx
psum accumulates the softmax denominators (no extra matmul).

Matmul cost on TRN2 = moving-column count (contraction/output width free), so
all operand layouts are chosen to minimize total moving columns:
Q 32.8k + K 8*8*kpad + V same + scores/ctx 16*sum(cap_j)*128 each + Wo 32.8k
cycles at 2.4 GHz (p-state held by a dense back-to-back PE stream).
"""

import math
from contextlib import ExitStack

import numpy as np
import ml_dtypes

import concourse.mybir as mybir
import concourse.tile as tile
from concourse import bacc
from concourse.bass_utils import run_bass_kernel_spmd

BF16 = mybir.dt.bfloat16
F32 = mybir.dt.float32

NEG_INF = -1e9
LN_EPS = 1e-6


class Cfg:
    def __init__(self, B=2, S=2048, D=1024, H=16, dh=64, kmax=None):
        self.B, self.S, self.D, self.H, self.dh = B, S, D, H, dh
        self.kmax = S if kmax is None else max(1, min(int(kmax), S))
        self.NC = 8                       # cores
        self.G = 4                        # cores per batch group
        self.RPC = S // self.G            # rows per core (512)
        self.NT = self.RPC // 128         # row-tiles per core (4)
        self.DC = D // 128                # contraction chunks (8)
        self.NP = H // 2                  # head pairs (8)
        self.KB = -(-self.kmax // 128)    # key chunks actually needed
        self.KPAD = self.KB * 128
        # slot j covers row tile 4*j+q (q = core quarter); the static chunk
        # cap must cover the deepest core (q=3)
        self.caps = [min(4 * j + 4, self.KB) for j in range(self.NT)]
        # (j, kb) positions where ANY core's causal diagonal can fall
        self.POS = [(j, kb) for j in range(self.NT)
                    for kb in range(self.caps[j])
                    if 4 * j <= kb <= 4 * j + 3]
        # runtime-detected LN specializations
        self.G1 = False
        self.B0 = False


def build_program(cfg: Cfg):
    nc = bacc.Bacc("TRN2", target_bir_lowering=False, debug=False,
                   num_devices=cfg.NC)

    D, dh = cfg.D, cfg.dh
    KB, KPAD, RPC, NT = cfg.KB, cfg.KPAD, cfg.RPC, cfg.NT

    xq = nc.dram_tensor("xq", [128, cfg.DC, RPC], BF16,
                        kind="ExternalInput").ap()
    xk = nc.dram_tensor("xk", [128, cfg.DC, KPAD], BF16,
                        kind="ExternalInput").ap()
    xv = nc.dram_tensor("xv", [KB, 128, cfg.DC, 128], BF16,
                        kind="ExternalInput").ap()
    wqT = nc.dram_tensor("wqT", [128, cfg.DC, D], BF16,
                         kind="ExternalInput").ap()
    wkT = nc.dram_tensor("wkT", [128, cfg.DC, D], BF16,
                         kind="ExternalInput").ap()
    wvT = nc.dram_tensor("wvT", [128, cfg.DC, D], BF16,
                         kind="ExternalInput").ap()
    woT = nc.dram_tensor("woT", [128, cfg.DC, D], BF16,
                         kind="ExternalInput").ap()
    bias_in = nc.dram_tensor("bias_in", [128, NT * KB], F32,
                             kind="ExternalInput").ap()
    trimask = nc.dram_tensor("trimask", [128, max(1, len(cfg.POS)), 8, 128],
                             BF16, kind="ExternalInput").ap()
    resid = nc.dram_tensor("resid", [128, NT, D], F32,
                           kind="ExternalInput").ap()
    gamma = nc.dram_tensor("gamma", [1, D], BF16, kind="ExternalInput").ap()
    beta = nc.dram_tensor("beta", [1, D], BF16, kind="ExternalInput").ap()
    out_shard = nc.dram_tensor("out_shard", [RPC, D], BF16,
                               kind="ExternalOutput").ap()

    with tile.TileContext(nc) as tc, ExitStack() as ctx:
        consts = ctx.enter_context(tc.tile_pool(name="consts", bufs=1))
        xin = ctx.enter_context(tc.tile_pool(name="xin", bufs=1))
        proj = ctx.enter_context(tc.tile_pool(name="proj", bufs=1))
        att = ctx.enter_context(tc.tile_pool(name="att", bufs=2))
        small = ctx.enter_context(tc.tile_pool(name="small", bufs=2))
        lnp = ctx.enter_context(tc.tile_pool(name="lnp", bufs=2))
        psum = ctx.enter_context(
            tc.tile_pool(name="psum", bufs=1, space="PSUM"))

        # ---- constants -----------------------------------------------------
        # weights ride the scalar engine's DMA queue (parallel to the sync
        # queue that streams activations); per-dc chunks so the first
        # accumulation chain can start after 1/8 of the weight transfer.
        # wk and wo share one slot (tag ring): wk is dead after the K
        # projection, long before Wo is needed, so wo's DMA just waits.
        wk_sb = consts.tile([128, cfg.DC, D], BF16, tag="wko", bufs=1,
                            name="wk_sb")
        wq_sb = consts.tile([128, cfg.DC, D], BF16)
        wv_sb = consts.tile([128, cfg.DC, D], BF16)
        for dc in range(cfg.DC):
            nc.scalar.dma_start(out=wk_sb[:, dc, :], in_=wkT[:, dc, :])
        for dc in range(cfg.DC):
            nc.scalar.dma_start(out=wq_sb[:, dc, :], in_=wqT[:, dc, :])
        for dc in range(cfg.DC):
            nc.scalar.dma_start(out=wv_sb[:, dc, :], in_=wvT[:, dc, :])

        bias_sb = consts.tile([128, NT * KB], F32)
        nc.sync.dma_start(out=bias_sb, in_=bias_in)
        # per-core multiplicative causal masks: at position i = (j, kb),
        # probs *= trimask[i] (all-ones unless this core's diagonal is at
        # that chunk, where it is the lower-triangle 0/1 mask).  Applied
        # POST-exp on bf16 SBUF so it stays off the scores->exp chain.
        tmask_sb = consts.tile([128, max(1, len(cfg.POS)), 8, 128], BF16)
        nc.sync.dma_start(out=tmask_sb, in_=trimask)

        # P3 constants (loaded later on the sync queue, behind the inputs)
        gamma_bc = beta_bc = None
        if not (cfg.G1 and cfg.B0):
            g_row = consts.tile([1, D], BF16)
            b_row = consts.tile([1, D], BF16)
            gamma_bc = consts.tile([128, D], BF16)
            beta_bc = consts.tile([128, D], BF16)
        eps_sb = consts.tile([128, 1], F32)
        nc.vector.memset(eps_sb, LN_EPS)
        res_sb = consts.tile([128, NT, D], F32)

        # ---- P1: projections ----------------------------------------------
        khT_sb = proj.tile([128, cfg.NP, KPAD], BF16)
        qhT_sb = proj.tile([128, cfg.NP, RPC], BF16)
        vh_sb = proj.tile([128, KB, cfg.H, dh + 1], BF16)
        nc.gpsimd.memset(vh_sb[:, :, :, dh:dh + 1], 1.0)

        # per-dc DMAs (contiguous per partition): the first K matmul only
        # waits on dc-chunk 0 (~160KB), not the whole transfer
        xk_sb = xin.tile([128, cfg.DC, KPAD], BF16)
        for dc in range(cfg.DC):
            nc.sync.dma_start(out=xk_sb[:, dc, :], in_=xk[:, dc, :])
        xq_sb = xin.tile([128, cfg.DC, RPC], BF16)
        for dc in range(cfg.DC):
            nc.sync.dma_start(out=xq_sb[:, dc, :], in_=xq[:, dc, :])
        xv_sb = []
        for kb in range(KB):
            t = xin.tile([128, cfg.DC, 128], BF16, tag="xv", bufs=KB,
                         name=f"xv{kb}")
            nc.sync.dma_start(out=t, in_=xv[kb])
            xv_sb.append(t)

        # remaining P3 constants transfer during P1/P2
        nc.sync.dma_start(out=res_sb, in_=resid)
        if gamma_bc is not None:
            nc.sync.dma_start(out=g_row, in_=gamma)
            nc.sync.dma_start(out=b_row, in_=beta)
            nc.gpsimd.partition_broadcast(gamma_bc, g_row)
            nc.gpsimd.partition_broadcast(beta_bc, b_row)

        copy_flip = [0]

        def pcopy(out_ap, in_ap):
            # alternate psum->sbuf copies between DVE and the scalar engine
            eng = nc.vector if copy_flip[0] % 2 == 0 else nc.scalar
            copy_flip[0] += 1
            if eng is nc.vector:
                eng.tensor_copy(out=out_ap, in_=in_ap)
            else:
                eng.copy(out=out_ap, in_=in_ap)

        def xw_proj(x_sb, w_sb, out_sb, width):
            # out[128 dims (pair), cols] = sum_dc w[:, dc, pair]^T? -- PE:
            # lhsT = w slice [128, 128], rhs = x [128, cols]
            for p in range(cfg.NP):
                for c0 in range(0, width, 512):
                    cw = min(512, width - c0)
                    ps = psum.tile([128, cw], F32, tag="ctx", bufs=4,
                                   name="ps_proj")
                    for dc in range(cfg.DC):
                        nc.tensor.matmul(
                            ps, w_sb[:, dc, p * 128:(p + 1) * 128],
                            x_sb[:, dc, c0:c0 + cw],
                            start=dc == 0, stop=dc == cfg.DC - 1)
                    pcopy(out_sb[:, p, c0:c0 + cw], ps)

        # K first (scores need it first), then Q, then V
        xw_proj(xk_sb, wk_sb, khT_sb, KPAD)
        xw_proj(xq_sb, wq_sb, qhT_sb, RPC)

        for kb in range(KB):
            for half in range(2):
                ps = psum.tile([128, 512], F32, tag="ctx", bufs=4,
                               name="ps_v")
                for dc in range(cfg.DC):
                    nc.tensor.matmul(
                        ps, xv_sb[kb][:, dc, :],
                        wv_sb[:, dc, half * 512:half * 512 + 512],
                        start=dc == 0, stop=dc == cfg.DC - 1)
                pcopy(
                    vh_sb[:, kb, 8 * half:8 * half + 8, 0:dh],
                    ps.rearrange("p (h e) -> p h e", e=dh))

        # wo reuses wk's SBUF slot (wk is dead now); DMA overlaps attention
        wo_sb = consts.tile([128, cfg.DC, D], BF16, tag="wko", bufs=1,
                            name="wo_sb")
        for dc in range(cfg.DC):
            nc.scalar.dma_start(out=wo_sb[:, dc, :], in_=woT[:, dc, :])

        # ---- P2: attention -------------------------------------------------
        ctxT_sb = proj.tile([128, cfg.NP, RPC], BF16)
        pos_idx = {pos: i for i, pos in enumerate(cfg.POS)}

        for j in range(NT):
            cap = cfg.caps[j]
            # 4 ctx psum banks for the slot: bank 2g+bb holds heads
            # (8g+4bb)..(8g+4bb+3), each 128 cols; row 64 = denominators
            ctx_b = [psum.tile([dh + 1, 512], F32, tag="ctx", bufs=4,
                               name=f"ctxps{b}") for b in range(4)]
            pend = []  # (kb, g, probs) awaiting their ctx matmuls
            for kb in range(cap):
                for g in range(2):
                    sc = psum.tile([128, 1024], F32, tag="sc", bufs=2,
                                   name="sc")
                    # HW constraint: matmuls whose stationary sits at
                    # different partition bases (PE row-tile 0 vs 64) must
                    # NOT write the same PSUM bank.  Even heads (base 0) go
                    # to bank 0 (cols 0-511), odd heads (base 64) to bank 1
                    # (cols 512-1023); one accumulation group per bank.
                    for hh in range(8):
                        h = 8 * g + hh
                        p, h2 = h // 2, h % 2
                        lo = 64 * h2
                        col = h2 * 512 + (hh // 2) * 128
                        nc.tensor.matmul(
                            sc[:, col:col + 128],
                            khT_sb[lo:lo + 64, p, kb * 128:(kb + 1) * 128],
                            qhT_sb[lo:lo + 64, p, j * 128:(j + 1) * 128],
                            start=hh < 2, stop=hh >= 6)
                    probs = att.tile([128, 1024], BF16, tag="pr", bufs=4,
                                     name="probs")
                    nc.scalar.activation(
                        out=probs, in_=sc,
                        func=mybir.ActivationFunctionType.Exp,
                        bias=bias_sb[:, j * KB + kb:j * KB + kb + 1],
                        scale=1.0 / math.sqrt(dh))
                    if (j, kb) in pos_idx:
                        i = pos_idx[(j, kb)]
                        nc.vector.tensor_mul(probs, probs, tmask_sb[:, i])
                    if len(pend) == 2:
                        okb, og, opr = pend.pop(0)
                        for hh in range(8):
                            h = 8 * og + hh
                            pcol = (hh % 2) * 512 + (hh // 2) * 128
                            nc.tensor.matmul(
                                ctx_b[2 * og + hh // 4]
                                [:, (hh % 4) * 128:(hh % 4) * 128 + 128],
                                vh_sb[:, okb, h, :],
                                opr[:, pcol:pcol + 128],
                                start=okb == 0 and hh % 4 == 0,
                                stop=okb == cap - 1 and hh % 4 == 3)
                    pend.append((kb, g, probs))
            for okb, og, opr in pend:
                for hh in range(8):
                    h = 8 * og + hh
                    pcol = (hh % 2) * 512 + (hh // 2) * 128
                    nc.tensor.matmul(
                        ctx_b[2 * og + hh // 4]
                        [:, (hh % 4) * 128:(hh % 4) * 128 + 128],
                        vh_sb[:, okb, h, :],
                        opr[:, pcol:pcol + 128],
                        start=okb == 0 and hh % 4 == 0,
                        stop=okb == cap - 1 and hh % 4 == 3)

            # divide by the denominators (psum row 64) and stage ctxT for Wo
            for b in range(4):
                den = small.tile([1, 512], F32, tag=f"den{b % 2}",
                                 bufs=2, name=f"den{b % 2}")
                nc.vector.tensor_copy(out=den, in_=ctx_b[b][dh:dh + 1, :])
                rec = small.tile([1, 512], F32, tag=f"rec{b % 2}",
                                 bufs=2, name=f"rec{b % 2}")
                nc.vector.reciprocal_approx_fast(rec, den)
                rbc = small.tile([64, 512], F32, tag=f"rbc{b % 2}", bufs=2,
                                 name=f"rbc{b % 2}")
                nc.gpsimd.partition_broadcast(rbc, rec)
                # bank b holds heads h0..h0+3 (pairs p0, p0+1); head parity
                # alternates partition halves of ctxT_sb
                h0 = 8 * (b // 2) + 4 * (b % 2)
                p0 = h0 // 2
                for par in range(2):
                    nc.vector.tensor_mul(
                        ctxT_sb[64 * par:64 * par + 64, p0:p0 + 2,
                                j * 128:(j + 1) * 128],
                        ctx_b[b][0:dh, :]
                        .rearrange("p (a c) -> p a c", a=4)[:, par::2, :],
                        rbc.rearrange("p (a c) -> p a c", a=4)
                        [:, par::2, :])

        # ---- P3: Wo + residual + LayerNorm ---------------------------------
        for rt in range(NT):
            pso = [psum.tile([128, 512], F32, tag="ctx", bufs=4,
                             name=f"pso{ns}") for ns in range(2)]
            for p in range(cfg.NP):
                for ns in range(2):
                    nc.tensor.matmul(
                        pso[ns], ctxT_sb[:, p, rt * 128:(rt + 1) * 128],
                        wo_sb[:, p, ns * 512:ns * 512 + 512],
                        start=p == 0, stop=p == cfg.NP - 1)
            x = lnp.tile([128, D], F32, tag="x")
            for ns in range(2):
                nc.vector.tensor_add(x[:, ns * 512:ns * 512 + 512], pso[ns],
                                     res_sb[:, rt, ns * 512:ns * 512 + 512])
            fmax = math.gcd(nc.vector.BN_STATS_FMAX, D)
            nsub = D // fmax
            stats = lnp.tile([128, nsub, nc.vector.BN_STATS_DIM], F32,
                             tag="stats")
            for sg in range(nsub):
                nc.vector.bn_stats(
                    out=stats[:, sg, :],
                    in_=x.rearrange("p (a b) -> p a b", a=nsub)[:, sg, :])
            mv = lnp.tile([128, nc.vector.BN_AGGR_DIM], F32, tag="mv")
            nc.vector.bn_aggr(out=mv, in_=stats)
            sd = lnp.tile([128, 1], F32, tag="sd")
            nc.scalar.activation(out=sd, in_=mv[:, 1:2],
                                 func=mybir.ActivationFunctionType.Sqrt,
                                 bias=eps_sb, scale=1.0)
            rstd = lnp.tile([128, 1], F32, tag="rstd")
            nc.vector.reciprocal_approx_fast(rstd, sd)
            out_sb = lnp.tile([128, D], BF16, tag="out_sb")
            if cfg.G1 and cfg.B0:
                nc.vector.tensor_scalar(
                    out=out_sb, in0=x, scalar1=mv[:, 0:1], scalar2=rstd,
                    op0=mybir.AluOpType.subtract, op1=mybir.AluOpType.mult)
            else:
                y = lnp.tile([128, D], BF16, tag="y")
                nc.vector.tensor_scalar(
                    out=y, in0=x, scalar1=mv[:, 0:1], scalar2=rstd,
                    op0=mybir.AluOpType.subtract, op1=mybir.AluOpType.mult)
                if cfg.B0:
                    nc.vector.tensor_mul(out_sb, y, gamma_bc)
                elif cfg.G1:
                    nc.vector.tensor_add(out_sb, y, beta_bc)
                else:
                    yg = lnp.tile([128, D], BF16, tag="yg")
                    nc.vector.tensor_mul(yg, y, gamma_bc)
                    nc.vector.tensor_add(out_sb, yg, beta_bc)
            nc.sync.dma_start(out=out_shard[rt * 128:(rt + 1) * 128, :],
                              in_=out_sb)

    nc.compile()
    return nc


def _tile_x(xT, dc=8):
    """[D, C] -> [128, dc, C] so the DMA is contiguous per partition."""
    d, c = xT.shape
    return np.ascontiguousarray(xT.reshape(dc, 128, c).transpose(1, 0, 2))


def _tile_w(wT):
    d, o = wT.shape
    return np.ascontiguousarray(
        wT.reshape(d // 128, 128, o).transpose(1, 0, 2))


def make_in_maps(cfg: Cfg, q, k, v, Wq, Wk, Wv, Wo, gamma, beta, sen_len):
    bf = ml_dtypes.bfloat16
    q = np.asarray(q, np.float32)
    k = np.asarray(k, np.float32)
    v = np.asarray(v, np.float32)
    wq_t = _tile_w(np.asarray(Wq, np.float32).T.astype(bf))
    wk_t = _tile_w(np.asarray(Wk, np.float32).T.astype(bf))
    wv_t = _tile_w(np.asarray(Wv, np.float32).T.astype(bf))
    wo_t = _tile_w(np.asarray(Wo, np.float32).T.astype(bf))
    g_row = np.asarray(gamma, np.float32).reshape(1, cfg.D).astype(bf)
    b_row = np.asarray(beta, np.float32).reshape(1, cfg.D).astype(bf)

    KB, KPAD, NT = cfg.KB, cfg.KPAD, cfg.NT
    per_batch = {}
    for b in range(cfg.B):
        kT = k[b, :KPAD, :].T.astype(bf)          # [D, KPAD]
        vT = v[b, :KPAD, :].T.astype(bf)
        xv = np.ascontiguousarray(
            vT.reshape(cfg.DC, 128, KB, 128).transpose(2, 1, 0, 3))
        per_batch[b] = (_tile_x(kT), xv)

    key_pos = np.arange(KPAD)
    in_maps = []
    for c in range(cfg.NC):
        b, qq = c // cfg.G, c % cfg.G
        tiles = [4 * j + qq for j in range(NT)]
        rows = np.concatenate(
            [np.arange(t * 128, (t + 1) * 128) for t in tiles])
        sl = int(np.asarray(sen_len)[b])

        bias = np.full((128, NT * KB), 0.0, np.float32)
        pad = np.where(key_pos < sl, 0.0, NEG_INF).astype(np.float32)
        for j in range(NT):
            d_j = tiles[j]  # diagonal chunk of this slot
            for kb in range(KB):
                col = pad[kb * 128:(kb + 1) * 128] if kb <= d_j \
                    else np.full(128, NEG_INF, np.float32)
                bias[:, j * KB + kb] = col
        # post-exp multiplicative masks: lower-triangle 0/1 at this core's
        # diagonal chunk positions, all-ones elsewhere
        bf = ml_dtypes.bfloat16
        tm = np.ones((128, max(1, len(cfg.POS)), 8, 128), bf)
        tril = (np.arange(128)[None, :] >= np.arange(128)[:, None])
        for i, (j, kb) in enumerate(cfg.POS):
            if kb == tiles[j]:
                tm[:, i, :, :] = tril.astype(bf)[:, None, :]

        xq_h = _tile_x(q[b][rows, :].T.astype(bf))
        res = np.ascontiguousarray(
            q[b][rows, :].reshape(NT, 128, cfg.D)
            .transpose(1, 0, 2).astype(np.float32))
        kT_t, xv_t = per_batch[b]
        in_maps.append({
            "xq": xq_h, "xk": kT_t, "xv": xv_t,
            "wqT": wq_t, "wkT": wk_t, "wvT": wv_t, "woT": wo_t,
            "bias_in": bias, "trimask": tm, "resid": res,
            "gamma": g_row, "beta": b_row,
        })
    return in_maps


def assemble_output(cfg: Cfg, results):
    out = np.empty((cfg.B, cfg.S, cfg.D), np.float32)
    for c in range(cfg.NC):
        b, qq = c // cfg.G, c % cfg.G
        shard = results[c]["out_shard"].astype(np.float32)
        for j in range(cfg.NT):
            t = 4 * j + qq
            out[b, t * 128:(t + 1) * 128, :] = shard[j * 128:(j + 1) * 128]
    return out


_PROGRAM_CACHE = {}


def _get_program(cfg: Cfg):
    key = (cfg.B, cfg.S, cfg.D, cfg.H, cfg.dh, cfg.KB, cfg.G1, cfg.B0)
    if key not in _PROGRAM_CACHE:
        _PROGRAM_CACHE[key] = build_program(cfg)
    return _PROGRAM_CACHE[key]


def run(cfg: Cfg, inputs: dict, trace: bool = False):
    cfg.G1 = bool(np.all(np.asarray(inputs["gamma"]) == 1.0))
    cfg.B0 = bool(np.all(np.asarray(inputs["beta"]) == 0.0))
    nc = _get_program(cfg)
    in_maps = make_in_maps(cfg, **inputs)
    res = run_bass_kernel_spmd(nc, in_maps, core_ids=list(range(cfg.NC)),
                               trace=trace)
    return assemble_output(cfg, res.results), res


def kernel(**inputs) -> np.ndarray:
    kmax = int(np.max(inputs["sen_len"]))
    cfg = Cfg(B=2, S=2048, D=1024, H=16, dh=64, kmax=kmax)
    out, _ = run(cfg, inputs)
    return out
